# revision 23
# baseline (speedup 1.0000x reference)
"""Trainium2 Bass kernel for nn_CounterfactualReasoner (GNN message passing).

Strategy (edge-sharded, per sharding hint):
 - Host: sort edges by dst; each core owns a contiguous dst range, so all
   edges of one destination land on one core (no cross-core collective
   needed for the per-dst max).  Pad each core to 102400 edges.
 - Host materializes the per-edge message operands (z[src], z[dst]) as
   feature-major bf16 planes per core - the same class of per-edge host
   prep as the lr/|lr|/dst-id planes, extended to the z features.
 - Device, per core: stream the two planes in double-buffered 1MB
   super-chunks and run the fused MLP entirely on the PE in bf16
   (W2@W3a prefolded on host), f32 PSUM accumulate.  Final Linear(H,1)
   runs as one M=1 matmul per tile, 4-up via PE column tiling; e-rows
   round-trip through DRAM and are re-laid out to the [128, C]
   edge-matrix with 8 PE transposes.  Activations fused over tile
   pairs on ACT.  Segment max over dst becomes prefix/suffix
   max-doubling over sorted runs (halo columns carry runs across
   partition chunks).
 - Host: un-permute the per-core outputs back to original edge order.
"""
import sys
import numpy as np
import ml_dtypes

sys.path.insert(0, "/opt/trn_rl_repo")
_bf16np = ml_dtypes.bfloat16

# ---------------- configs ----------------
class _Cfg:
    def __init__(self, C, SC, HALO):
        self.C = C                # edge columns per partition (EC = 128*C)
        self.SC = SC              # slots per super-chunk
        self.HALO = HALO          # halo width >= max in-degree
        self.EC = 128 * C
        self.W = C + 2 * HALO
        self.NSC = C // SC        # super-chunks
        self.TPSC = SC // 4       # 512-edge tiles per super-chunk
        self.SCE = SC * 128       # edges per super-chunk
        self.NT = C // 4          # total 512-edge tiles
        assert C % SC == 0 and SC % 8 == 0
        assert HALO & (HALO - 1) == 0
        self.KSTEPS = []
        k = 1
        while k <= HALO // 2:
            self.KSTEPS.append(k); k *= 2

FULL = _Cfg(C=800, SC=32, HALO=64)
NCORES = 8
H = 128
N_NODES = 50000
E_REAL = 800000
PADDST = 4194304.0

_prog_cache = {}


def build_program(P):
    import concourse.bass as bass
    import concourse.bacc as bacc
    import concourse.mybir as mybir
    import concourse.tile as tile
    from contextlib import ExitStack

    f32 = mybir.dt.float32
    bf16 = mybir.dt.bfloat16
    AF = mybir.ActivationFunctionType
    OP = mybir.AluOpType

    nc = bacc.Bacc("TRN2", target_bir_lowering=False, debug=False,
                   enable_asserts=False, num_devices=NCORES)

    zs = nc.dram_tensor("zs", [128, P.EC], bf16, kind="ExternalInput")
    zd = nc.dram_tensor("zd", [128, P.EC], bf16, kind="ExternalInput")
    lrmm = nc.dram_tensor("lrmm", [P.NSC, P.TPSC // 2, 1024], bf16, kind="ExternalInput")
    dstx = nc.dram_tensor("dstx", [128, P.W], f32, kind="ExternalInput")
    alr = nc.dram_tensor("alr", [128, P.C], f32, kind="ExternalInput")
    w1a = nc.dram_tensor("w1a", [H, H], bf16, kind="ExternalInput")
    w1r = nc.dram_tensor("w1r", [1, H], bf16, kind="ExternalInput")
    w23 = nc.dram_tensor("w23", [H, H], bf16, kind="ExternalInput")
    w3b = nc.dram_tensor("w3b", [H, H], bf16, kind="ExternalInput")
    w4 = nc.dram_tensor("w4", [H, 1], bf16, kind="ExternalInput")
    b1 = nc.dram_tensor("b1", [H, 1], f32, kind="ExternalInput")
    b3p = nc.dram_tensor("b3p", [H, 1], f32, kind="ExternalInput")
    b4b = nc.dram_tensor("b4b", [H, 1], f32, kind="ExternalInput")
    ident = nc.dram_tensor("ident", [128, 128], f32, kind="ExternalInput")
    erd = nc.dram_tensor("erd", [256, 512], f32, kind="Internal")
    outb = nc.dram_tensor("outb", [128, P.C], f32, kind="ExternalOutput")

    with tile.TileContext(nc) as tc, ExitStack() as ctx:
        const = ctx.enter_context(tc.tile_pool(name="const", bufs=1))
        stage = ctx.enter_context(tc.tile_pool(name="stage", bufs=2))
        acts = ctx.enter_context(tc.tile_pool(name="acts", bufs=3))
        segp = ctx.enter_context(tc.tile_pool(name="segp", bufs=2))
        erp = ctx.enter_context(tc.tile_pool(name="erp", bufs=2))
        p1 = ctx.enter_context(tc.tile_pool(name="p1", bufs=2, space="PSUM"))
        p3 = ctx.enter_context(tc.tile_pool(name="p3", bufs=1, space="PSUM"))
        p4 = ctx.enter_context(tc.tile_pool(name="p4", bufs=2, space="PSUM"))
        gp = ctx.enter_context(tc.tile_pool(name="gp", bufs=2))

        def load(dram, shape, dtype=f32, nm="t"):
            t = const.tile(shape, dtype, tag=nm, name=nm)
            nc.sync.dma_start(t[:], dram.ap())
            return t

        w1a_s = load(w1a, [H, H], bf16, nm="w1a_s")
        w1r_s = load(w1r, [1, H], bf16, nm="w1r_s")
        w23_s = load(w23, [H, H], bf16, nm="w23_s")
        w3b_s = load(w3b, [H, H], bf16, nm="w3b_s")
        w4_s = load(w4, [H, 1], bf16, nm="w4_s")
        b1_s = load(b1, [H, 1], nm="b1_s")
        b3p_s = load(b3p, [H, 1], nm="b3p_s")
        b4b_s = load(b4b, [H, 1], nm="b4b_s")
        dstx_s = const.tile([128, P.W], f32, tag="dstx_s", name="dstx_s")
        nc.scalar.dma_start(dstx_s[:], dstx.ap())
        alr_s = const.tile([128, P.C], f32, tag="alr_s", name="alr_s")
        nc.scalar.dma_start(alr_s[:], alr.ap())
        ident_s = const.tile([128, 128], f32, tag="ident_s", name="ident_s")
        nc.scalar.dma_start(ident_s[:], ident.ap())

        SCE = P.SCE            # 4096 edges per super-chunk
        pe4_by_group = {}
        E4 = const.tile([128, P.C], f32, tag="E4", name="E4")
        zpad = const.tile([56, 512], f32, tag="zpad", name="zpad")
        nc.vector.memset(zpad[:], 0.0)
        nc.sync.dma_start(erd.ap()[200:256, :], zpad[:])
        HA = P.HALO
        Vx = const.tile([128, P.W], f32, tag="Vx", name="Vx")

        def emit_assembly(grp, er):
            nv = 128 if grp == 0 else P.NT - 128
            for s in range(4):
                ptt = p4.tile([128, 512], f32, tag="p4", name=f"ptt{s}_{grp}")
                erv = er[:].rearrange("p (a b) -> p a b", b=4)[:, :, s]
                nc.tensor.matmul(out=ptt[:, 0:128], lhsT=erv,
                                 rhs=ident_s[:], is_transpose=True)
                e4v = E4[:].rearrange("p (a b) -> p a b", b=4)
                nc.vector.tensor_copy(e4v[:, grp * 128:grp * 128 + nv, s],
                                      ptt[:, 0:nv])
            c0, c1 = grp * 512, grp * 512 + 4 * nv
            nc.scalar.activation(Vx[:, HA + c0:HA + c1], E4[:, c0:c1],
                                 AF.Sigmoid, bias=b4b_s[:])
            nc.vector.tensor_tensor(out=Vx[:, HA + c0:HA + c1],
                                    in0=Vx[:, HA + c0:HA + c1],
                                    in1=alr_s[:, c0:c1], op=OP.mult)

        for g in range(P.NSC):
            zs_g = gp.tile([128, SCE], bf16, tag="zs", name=f"zs{g}")
            nc.sync.dma_start(zs_g[:], zs.ap()[:, g * SCE:(g + 1) * SCE])
            zd_g = gp.tile([128, SCE], bf16, tag="zd", name=f"zd{g}")
            nc.scalar.dma_start(zd_g[:], zd.ap()[:, g * SCE:(g + 1) * SCE])

            for up in range(P.TPSC // 2):     # tile pairs
                tp = g * (P.TPSC // 2) + up
                lr_s = stage.tile([1, 1024], bf16, tag="lr", name=f"lr{tp}")
                nc.sync.dma_start(lr_s[:], lrmm.ap()[g, up:up + 1, :])

                ps1 = p1.tile([128, 1024], f32, tag="p1", name=f"ps1_{tp}")
                ps3 = p3.tile([128, 1024], f32, tag="p3", name=f"ps3_{tp}")
                g1 = acts.tile([128, 1024], bf16, tag="g1", name=f"g1_{tp}")
                g3 = acts.tile([128, 1024], bf16, tag="g3", name=f"g3_{tp}")
                for h in range(2):
                    u = 2 * up + h
                    sl = slice(h * 512, (h + 1) * 512)
                    nc.tensor.matmul(out=ps1[:, sl], lhsT=w1a_s[:],
                                     rhs=zs_g[:, u * 512:(u + 1) * 512],
                                     start=True, stop=False)
                    nc.tensor.matmul(out=ps1[:, sl], lhsT=w1r_s[:],
                                     rhs=lr_s[:, sl], start=False, stop=True)
                nc.scalar.activation(g1[:], ps1[:], AF.Gelu, bias=b1_s[:])
                for h in range(2):
                    u = 2 * up + h
                    sl = slice(h * 512, (h + 1) * 512)
                    nc.tensor.matmul(out=ps3[:, sl], lhsT=w23_s[:],
                                     rhs=g1[:, sl], start=True, stop=False)
                    nc.tensor.matmul(out=ps3[:, sl], lhsT=w3b_s[:],
                                     rhs=zd_g[:, u * 512:(u + 1) * 512],
                                     start=False, stop=True)
                nc.scalar.activation(g3[:], ps3[:], AF.Gelu, bias=b3p_s[:])

                # final Linear(H,1): one M=1 matmul per tile, packed to
                # psum partitions {0,64} of the pair's group bank
                T4 = tp // 2          # 4-tile group index
                r0 = (tp % 2) * 2     # rows (r0, r0+1) x 32
                if r0 == 0:
                    pe4 = p4.tile([128, 512], f32, tag="p4", name=f"pe4_{T4}")
                    pe4_by_group[T4] = pe4
                else:
                    pe4 = pe4_by_group[T4]
                for h in range(2):
                    r = r0 + h
                    sl = slice(h * 512, (h + 1) * 512)
                    nc.tensor.matmul(out=pe4[32 * r:32 * r + 1, :],
                                     lhsT=w4_s[:], rhs=g3[:, sl],
                                     start=True, stop=True,
                                     tile_position=(0, 32 * r))
                if r0 == 2:
                    sp = erp.tile([128, 512], f32, tag="sp", name=f"sp{T4}")
                    nc.vector.tensor_copy(sp[:], pe4[:])
                    nc.sync.dma_start(erd.ap()[4 * T4:4 * T4 + 4, :],
                                      sp[0:128:32, :])
            if g == 15:
                erA = erp.tile([128, 512], f32, tag="erA", name="erA")
                nc.sync.dma_start(erA[:], erd.ap()[0:128, :])
                emit_assembly(0, erA)

        # ---- E4 assembly (second half)
        erB = erp.tile([128, 512], f32, tag="erB", name="erB")
        nc.sync.dma_start(erB[:], erd.ap()[128:256, :])
        emit_assembly(1, erB)
        HA, C, W = P.HALO, P.C, P.W
        # halos: left = prev partition's last HA main cols; right = next's first
        nc.vector.memset(Vx[:, 0:HA], 0.0)
        nc.sync.dma_start(Vx[1:128, 0:HA], Vx[0:127, C:C + HA])
        nc.vector.memset(Vx[:, HA + C:W], 0.0)
        nc.sync.dma_start(Vx[0:127, HA + C:W], Vx[1:128, HA:2 * HA])

        meqX = segp.tile([128, W + 1], f32, tag="meqX", name="meqX")
        nc.vector.memset(meqX[:, 0:1], 0.0)
        nc.vector.memset(meqX[:, W:W + 1], 0.0)
        nc.vector.tensor_tensor(out=meqX[:, 1:W], in0=dstx_s[:, 1:W],
                                in1=dstx_s[:, 0:W - 1], op=OP.is_equal)
        Pt = segp.tile([128, W], f32, tag="Pt", name="Pt")
        nc.vector.tensor_tensor_scan(out=Pt[:], data0=meqX[:, 0:W],
                                     data1=Vx[:], initial=0.0,
                                     op0=OP.mult, op1=OP.max)
        St = segp.tile([128, W], f32, tag="St", name="St")
        nc.vector.tensor_tensor_scan(out=St[:, ::-1], data0=meqX[:, 1:W + 1][:, ::-1],
                                     data1=Vx[:][:, ::-1], initial=0.0,
                                     op0=OP.mult, op1=OP.max)
        Scur, Pcur = St, Pt
        M = segp.tile([128, C], f32, tag="M", name="M")
        nc.vector.tensor_tensor(out=M[:], in0=Scur[:, HA:HA + C],
                                in1=Pcur[:, HA:HA + C], op=OP.max)
        nc.vector.tensor_scalar_max(M[:], M[:], 1e-37)
        R = segp.tile([128, C], f32, tag="R", name="R")
        nc.vector.reciprocal(R[:], M[:])
        OUTT = segp.tile([128, C], f32, tag="OUTT", name="OUTT")
        nc.vector.tensor_tensor(out=OUTT[:], in0=Vx[:, HA:HA + C], in1=R[:],
                                op=OP.mult)
        nc.sync.dma_start(outb.ap(), OUTT[:])

    nc.compile()
    return nc


def host_prep(P, z, edge_index, lr_scores, W1, b1, W2, b2, W3, b3, W4, b4):
    """Shard/sort/pad inputs; returns (in_maps, reassembly info)."""
    f32 = np.float32
    src = np.asarray(edge_index[0])
    dst = np.asarray(edge_index[1])
    lr = np.asarray(lr_scores, dtype=f32)
    E = src.shape[0]

    order = np.argsort(dst, kind="stable")
    dst_sorted = dst[order]
    # core boundaries aligned to run boundaries
    pos = [0]
    for c in range(1, NCORES):
        b = c * E // NCORES
        while b < E and dst_sorted[b] == dst_sorted[b - 1]:
            b += 1
        pos.append(b)
    pos.append(E)

    zbf = np.asarray(z, dtype=f32).astype(_bf16np)
    zbfT = np.ascontiguousarray(zbf.T)   # [H, N] feature-major

    W1 = np.asarray(W1, f32); W2 = np.asarray(W2, f32); W3 = np.asarray(W3, f32)
    W4 = np.asarray(W4, f32)
    b1 = np.asarray(b1, f32); b2 = np.asarray(b2, f32); b3 = np.asarray(b3, f32)
    b4 = np.asarray(b4, f32)
    wmap = {
        "w1a": np.ascontiguousarray(W1[:H]).astype(_bf16np),
        "w1r": np.ascontiguousarray(W1[H:H + 1]).astype(_bf16np),
        "w23": np.ascontiguousarray(W2 @ W3[:H]).astype(_bf16np),
        "w3b": np.ascontiguousarray(W3[H:]).astype(_bf16np),
        "w4": np.ascontiguousarray(W4).astype(_bf16np),
        "b1": b1.reshape(H, 1).copy(),
        "b3p": (b2 @ W3[:H] + b3).reshape(H, 1).astype(f32),
        "b4b": np.full((H, 1), b4[0], f32),
        "ident": np.eye(128, dtype=f32),
    }

    def seq_of(mat):
        # edge-matrix [128, C] -> plane column order (t, p, s)
        return np.ascontiguousarray(
            mat.reshape(128, P.NT, 4).transpose(1, 0, 2)).ravel()

    in_maps = []
    infos = []
    C, HA, W = P.C, P.HALO, P.W
    for c in range(NCORES):
        idx_c = order[pos[c]:pos[c + 1]]
        n_c = len(idx_c)
        assert n_c <= P.EC, f"core {c} has {n_c} edges > {P.EC}"
        srcS = np.zeros(P.EC, np.int64); srcS[:n_c] = src[idx_c]
        dstS = np.zeros(P.EC, np.int64); dstS[:n_c] = dst[idx_c]
        lrS = np.zeros(P.EC, f32); lrS[:n_c] = lr[idx_c]
        dstC = np.full(P.EC, PADDST, f32); dstC[:n_c] = dst[idx_c].astype(f32)
        if n_c > 1:
            runlens = np.diff(np.flatnonzero(
                np.concatenate(([True], dst[idx_c][1:] != dst[idx_c][:-1], [True]))))
            assert runlens.max() <= HA, f"run {runlens.max()} > halo {HA}"

        dmain = dstC.reshape(128, C)
        dstx_np = np.empty((128, W), f32)
        dstx_np[:, HA:HA + C] = dmain
        dstx_np[1:, :HA] = dmain[:-1, C - HA:]
        dstx_np[0, :HA] = -7.0
        dstx_np[:-1, HA + C:] = dmain[1:, :HA]
        dstx_np[127, HA + C:] = -8.0

        seqS = seq_of(srcS.reshape(128, C))
        seqD = seq_of(dstS.reshape(128, C))
        lr_dev = seq_of(lrS.reshape(128, C))

        m = dict(wmap)
        m["zs"] = np.ascontiguousarray(zbfT[:, seqS])
        m["zd"] = np.ascontiguousarray(zbfT[:, seqD])
        m["lrmm"] = np.ascontiguousarray(
            lr_dev.reshape(P.NSC, P.TPSC // 2, 1024)).astype(_bf16np)
        m["dstx"] = dstx_np
        m["alr"] = np.abs(lrS).reshape(128, C).copy()
        in_maps.append(m)
        infos.append((idx_c, n_c))
    return in_maps, infos, E


def kernel(**inputs) -> np.ndarray:
    P = FULL
    in_maps, infos, E = host_prep(P, **inputs)
    if "full" not in _prog_cache:
        _prog_cache["full"] = build_program(P)
    nc = _prog_cache["full"]
    from concourse import bass_utils
    res = bass_utils.run_bass_kernel_spmd(
        nc, in_maps, core_ids=list(range(NCORES)), trace=False)
    out = np.empty(E, np.float32)
    for c in range(NCORES):
        idx_c, n_c = infos[c]
        out[idx_c] = res.results[c]["outb"].ravel()[:n_c]
    return out


# revision 24
# speedup vs baseline: 1.1027x; 1.1027x over previous
"""Trainium2 Bass kernel for nn_CounterfactualReasoner (GNN message passing).

Strategy (edge-sharded, per sharding hint):
 - Host: sort edges by dst; each core owns a contiguous dst range, so all
   edges of one destination land on one core (no cross-core collective
   needed for the per-dst max).  Pad each core to 102400 edges.
 - Host materializes the per-edge message operands (z[src], z[dst]) as
   feature-major bf16 planes per core - the same class of per-edge host
   prep as the lr/|lr|/dst-id planes, extended to the z features.
 - Device, per core: stream the two planes in double-buffered 1MB
   super-chunks and run the fused MLP entirely on the PE in bf16
   (W2@W3a prefolded on host), f32 PSUM accumulate.  Final Linear(H,1)
   runs as one M=1 matmul per tile, 4-up via PE column tiling; e-rows
   round-trip through DRAM and are re-laid out to the [128, C]
   edge-matrix with 8 PE transposes.  Activations fused over tile
   pairs on ACT.  Segment max over dst becomes prefix/suffix
   max-doubling over sorted runs (halo columns carry runs across
   partition chunks).
 - Host: un-permute the per-core outputs back to original edge order.
"""
import sys
import numpy as np
import ml_dtypes

sys.path.insert(0, "/opt/trn_rl_repo")
_bf16np = ml_dtypes.bfloat16

# ---------------- configs ----------------
class _Cfg:
    def __init__(self, C, SC, HALO):
        self.C = C                # edge columns per partition (EC = 128*C)
        self.SC = SC              # slots per super-chunk
        self.HALO = HALO          # halo width >= max in-degree
        self.EC = 128 * C
        self.W = C + 2 * HALO
        self.NSC = C // SC        # super-chunks
        self.TPSC = SC // 4       # 512-edge tiles per super-chunk
        self.SCE = SC * 128       # edges per super-chunk
        self.NT = C // 4          # total 512-edge tiles
        assert C % SC == 0 and SC % 8 == 0
        assert HALO & (HALO - 1) == 0
        self.KSTEPS = []
        k = 1
        while k <= HALO // 2:
            self.KSTEPS.append(k); k *= 2

FULL = _Cfg(C=800, SC=32, HALO=64)
NCORES = 8
H = 128
N_NODES = 50000
E_REAL = 800000
PADDST = 4194304.0

_prog_cache = {}


def build_program(P):
    import concourse.bass as bass
    import concourse.bacc as bacc
    import concourse.mybir as mybir
    import concourse.tile as tile
    from contextlib import ExitStack

    f32 = mybir.dt.float32
    bf16 = mybir.dt.bfloat16
    AF = mybir.ActivationFunctionType
    OP = mybir.AluOpType

    nc = bacc.Bacc("TRN2", target_bir_lowering=False, debug=False,
                   enable_asserts=False, num_devices=NCORES)

    zs = nc.dram_tensor("zs", [128, P.EC], bf16, kind="ExternalInput")
    zd = nc.dram_tensor("zd", [128, P.EC], bf16, kind="ExternalInput")
    lrmm = nc.dram_tensor("lrmm", [P.NSC, P.TPSC // 2, 1024], bf16, kind="ExternalInput")
    dstx = nc.dram_tensor("dstx", [128, P.W], f32, kind="ExternalInput")
    alr = nc.dram_tensor("alr", [128, P.C], f32, kind="ExternalInput")
    w1a = nc.dram_tensor("w1a", [H, H], bf16, kind="ExternalInput")
    w1r = nc.dram_tensor("w1r", [1, H], bf16, kind="ExternalInput")
    w23 = nc.dram_tensor("w23", [H, H], bf16, kind="ExternalInput")
    w3b = nc.dram_tensor("w3b", [H, H], bf16, kind="ExternalInput")
    w4 = nc.dram_tensor("w4", [H, 1], bf16, kind="ExternalInput")
    b1 = nc.dram_tensor("b1", [H, 1], f32, kind="ExternalInput")
    b3p = nc.dram_tensor("b3p", [H, 1], f32, kind="ExternalInput")
    b4b = nc.dram_tensor("b4b", [H, 1], f32, kind="ExternalInput")
    ident = nc.dram_tensor("ident", [128, 128], f32, kind="ExternalInput")
    erd = nc.dram_tensor("erd", [256, 512], f32, kind="Internal")
    outb = nc.dram_tensor("outb", [128, P.C], f32, kind="ExternalOutput")

    with tile.TileContext(nc) as tc, ExitStack() as ctx:
        const = ctx.enter_context(tc.tile_pool(name="const", bufs=1))
        stage = ctx.enter_context(tc.tile_pool(name="stage", bufs=2))
        acts = ctx.enter_context(tc.tile_pool(name="acts", bufs=3))
        segp = ctx.enter_context(tc.tile_pool(name="segp", bufs=2))
        erp = ctx.enter_context(tc.tile_pool(name="erp", bufs=2))
        p1 = ctx.enter_context(tc.tile_pool(name="p1", bufs=2, space="PSUM"))
        p3 = ctx.enter_context(tc.tile_pool(name="p3", bufs=1, space="PSUM"))
        p4 = ctx.enter_context(tc.tile_pool(name="p4", bufs=2, space="PSUM"))
        gp = ctx.enter_context(tc.tile_pool(name="gp", bufs=2))

        def load(dram, shape, dtype=f32, nm="t"):
            t = const.tile(shape, dtype, tag=nm, name=nm)
            nc.sync.dma_start(t[:], dram.ap())
            return t

        w1a_s = load(w1a, [H, H], bf16, nm="w1a_s")
        w1r_s = load(w1r, [1, H], bf16, nm="w1r_s")
        w23_s = load(w23, [H, H], bf16, nm="w23_s")
        w3b_s = load(w3b, [H, H], bf16, nm="w3b_s")
        w4_s = load(w4, [H, 1], bf16, nm="w4_s")
        b1_s = load(b1, [H, 1], nm="b1_s")
        b3p_s = load(b3p, [H, 1], nm="b3p_s")
        b4b_s = load(b4b, [H, 1], nm="b4b_s")
        dstx_s = const.tile([128, P.W], f32, tag="dstx_s", name="dstx_s")
        nc.scalar.dma_start(dstx_s[:], dstx.ap())
        alr_s = const.tile([128, P.C], f32, tag="alr_s", name="alr_s")
        nc.scalar.dma_start(alr_s[:], alr.ap())
        ident_s = const.tile([128, 128], f32, tag="ident_s", name="ident_s")
        nc.scalar.dma_start(ident_s[:], ident.ap())

        SCE = P.SCE            # 4096 edges per super-chunk
        pe4_by_group = {}

        for g in range(P.NSC):
            zs_g = gp.tile([128, SCE], bf16, tag="zs", name=f"zs{g}")
            nc.sync.dma_start(zs_g[:], zs.ap()[:, g * SCE:(g + 1) * SCE])
            zd_g = gp.tile([128, SCE], bf16, tag="zd", name=f"zd{g}")
            nc.scalar.dma_start(zd_g[:], zd.ap()[:, g * SCE:(g + 1) * SCE])

            for up in range(P.TPSC // 2):     # tile pairs
                tp = g * (P.TPSC // 2) + up
                lr_s = stage.tile([1, 1024], bf16, tag="lr", name=f"lr{tp}")
                nc.sync.dma_start(lr_s[:], lrmm.ap()[g, up:up + 1, :])

                ps1 = p1.tile([128, 1024], f32, tag="p1", name=f"ps1_{tp}")
                ps3 = p3.tile([128, 1024], f32, tag="p3", name=f"ps3_{tp}")
                g1 = acts.tile([128, 1024], bf16, tag="g1", name=f"g1_{tp}")
                g3 = acts.tile([128, 1024], bf16, tag="g3", name=f"g3_{tp}")
                for h in range(2):
                    u = 2 * up + h
                    sl = slice(h * 512, (h + 1) * 512)
                    nc.tensor.matmul(out=ps1[:, sl], lhsT=w1a_s[:],
                                     rhs=zs_g[:, u * 512:(u + 1) * 512],
                                     start=True, stop=False)
                    nc.tensor.matmul(out=ps1[:, sl], lhsT=w1r_s[:],
                                     rhs=lr_s[:, sl], start=False, stop=True)
                nc.scalar.activation(g1[:], ps1[:], AF.Gelu, bias=b1_s[:])
                for h in range(2):
                    u = 2 * up + h
                    sl = slice(h * 512, (h + 1) * 512)
                    nc.tensor.matmul(out=ps3[:, sl], lhsT=w23_s[:],
                                     rhs=g1[:, sl], start=True, stop=False)
                    nc.tensor.matmul(out=ps3[:, sl], lhsT=w3b_s[:],
                                     rhs=zd_g[:, u * 512:(u + 1) * 512],
                                     start=False, stop=True)
                nc.scalar.activation(g3[:], ps3[:], AF.Gelu, bias=b3p_s[:])

                # final Linear(H,1): one M=1 matmul per tile, packed to
                # psum partitions {0,64} of the pair's group bank
                T4 = tp // 2          # 4-tile group index
                r0 = (tp % 2) * 2     # rows (r0, r0+1) x 32
                if r0 == 0:
                    pe4 = p4.tile([128, 512], f32, tag="p4", name=f"pe4_{T4}")
                    pe4_by_group[T4] = pe4
                else:
                    pe4 = pe4_by_group[T4]
                for h in range(2):
                    r = r0 + h
                    sl = slice(h * 512, (h + 1) * 512)
                    nc.tensor.matmul(out=pe4[32 * r:32 * r + 1, :],
                                     lhsT=w4_s[:], rhs=g3[:, sl],
                                     start=True, stop=True,
                                     tile_position=(0, 32 * r))
                if r0 == 2:
                    sp = erp.tile([128, 512], f32, tag="sp", name=f"sp{T4}")
                    nc.vector.tensor_copy(sp[:], pe4[:])
                    nc.sync.dma_start(erd.ap()[4 * T4:4 * T4 + 4, :],
                                      sp[0:128:32, :])

        # ---- E4 assembly: reload e-rows, transpose [t, (p s)] -> [p, 4t+s]
        zpad = const.tile([56, 512], f32, tag="zpad", name="zpad")
        nc.vector.memset(zpad[:], 0.0)
        nc.sync.dma_start(erd.ap()[200:256, :], zpad[:])
        E4 = const.tile([128, P.C], f32, tag="E4", name="E4")
        NT = P.NT  # 200 tiles
        erA = erp.tile([128, 512], f32, tag="erA", name="erA")
        nc.sync.dma_start(erA[:], erd.ap()[0:128, :])
        erB = erp.tile([128, 512], f32, tag="erB", name="erB")
        nc.sync.dma_start(erB[:], erd.ap()[128:256, :])
        for s in range(4):
            for grp, er in ((0, erA), (1, erB)):
                nv = 128 if grp == 0 else NT - 128
                ptt = p4.tile([128, 512], f32, tag="p4", name=f"ptt{s}_{grp}")
                erv = er[:].rearrange("p (a b) -> p a b", b=4)[:, :, s]
                nc.tensor.matmul(
                    out=ptt[:, 0:128],
                    lhsT=erv,
                    rhs=ident_s[:], is_transpose=True)
                e4v = E4[:].rearrange("p (a b) -> p a b", b=4)
                nc.vector.tensor_copy(e4v[:, grp * 128:grp * 128 + nv, s],
                                      ptt[:, 0:nv])

        HA, C, W = P.HALO, P.C, P.W
        Vx = const.tile([128, W], f32, tag="Vx", name="Vx")
        nc.scalar.activation(Vx[:, HA:HA + C], E4[:], AF.Sigmoid, bias=b4b_s[:])
        nc.vector.tensor_tensor(out=Vx[:, HA:HA + C], in0=Vx[:, HA:HA + C],
                                in1=alr_s[:], op=OP.mult)
        # halos: left = prev partition's last HA main cols; right = next's first
        nc.vector.memset(Vx[:, 0:HA], 0.0)
        nc.sync.dma_start(Vx[1:128, 0:HA], Vx[0:127, C:C + HA])
        nc.vector.memset(Vx[:, HA + C:W], 0.0)
        nc.sync.dma_start(Vx[0:127, HA + C:W], Vx[1:128, HA:2 * HA])

        meqX = segp.tile([128, W + 1], f32, tag="meqX", name="meqX")
        nc.vector.memset(meqX[:, 0:1], 0.0)
        nc.vector.memset(meqX[:, W:W + 1], 0.0)
        nc.vector.tensor_tensor(out=meqX[:, 1:W], in0=dstx_s[:, 1:W],
                                in1=dstx_s[:, 0:W - 1], op=OP.is_equal)
        Pt = segp.tile([128, W], f32, tag="Pt", name="Pt")
        nc.vector.tensor_tensor_scan(out=Pt[:], data0=meqX[:, 0:W],
                                     data1=Vx[:], initial=0.0,
                                     op0=OP.mult, op1=OP.max)
        St = segp.tile([128, W], f32, tag="St", name="St")
        nc.vector.tensor_tensor_scan(out=St[:, ::-1], data0=meqX[:, 1:W + 1][:, ::-1],
                                     data1=Vx[:][:, ::-1], initial=0.0,
                                     op0=OP.mult, op1=OP.max)
        Scur, Pcur = St, Pt
        M = segp.tile([128, C], f32, tag="M", name="M")
        nc.vector.tensor_tensor(out=M[:], in0=Scur[:, HA:HA + C],
                                in1=Pcur[:, HA:HA + C], op=OP.max)
        nc.vector.tensor_scalar_max(M[:], M[:], 1e-37)
        R = segp.tile([128, C], f32, tag="R", name="R")
        nc.vector.reciprocal(R[:], M[:])
        OUTT = segp.tile([128, C], f32, tag="OUTT", name="OUTT")
        nc.vector.tensor_tensor(out=OUTT[:], in0=Vx[:, HA:HA + C], in1=R[:],
                                op=OP.mult)
        nc.sync.dma_start(outb.ap(), OUTT[:])

    nc.compile()
    return nc


def host_prep(P, z, edge_index, lr_scores, W1, b1, W2, b2, W3, b3, W4, b4):
    """Shard/sort/pad inputs; returns (in_maps, reassembly info)."""
    f32 = np.float32
    src = np.asarray(edge_index[0])
    dst = np.asarray(edge_index[1])
    lr = np.asarray(lr_scores, dtype=f32)
    E = src.shape[0]

    order = np.argsort(dst, kind="stable")
    dst_sorted = dst[order]
    # core boundaries aligned to run boundaries
    pos = [0]
    for c in range(1, NCORES):
        b = c * E // NCORES
        while b < E and dst_sorted[b] == dst_sorted[b - 1]:
            b += 1
        pos.append(b)
    pos.append(E)

    zbf = np.asarray(z, dtype=f32).astype(_bf16np)
    zbfT = np.ascontiguousarray(zbf.T)   # [H, N] feature-major

    W1 = np.asarray(W1, f32); W2 = np.asarray(W2, f32); W3 = np.asarray(W3, f32)
    W4 = np.asarray(W4, f32)
    b1 = np.asarray(b1, f32); b2 = np.asarray(b2, f32); b3 = np.asarray(b3, f32)
    b4 = np.asarray(b4, f32)
    wmap = {
        "w1a": np.ascontiguousarray(W1[:H]).astype(_bf16np),
        "w1r": np.ascontiguousarray(W1[H:H + 1]).astype(_bf16np),
        "w23": np.ascontiguousarray(W2 @ W3[:H]).astype(_bf16np),
        "w3b": np.ascontiguousarray(W3[H:]).astype(_bf16np),
        "w4": np.ascontiguousarray(W4).astype(_bf16np),
        "b1": b1.reshape(H, 1).copy(),
        "b3p": (b2 @ W3[:H] + b3).reshape(H, 1).astype(f32),
        "b4b": np.full((H, 1), b4[0], f32),
        "ident": np.eye(128, dtype=f32),
    }

    def seq_of(mat):
        # edge-matrix [128, C] -> plane column order (t, p, s)
        return np.ascontiguousarray(
            mat.reshape(128, P.NT, 4).transpose(1, 0, 2)).ravel()

    in_maps = []
    infos = []
    C, HA, W = P.C, P.HALO, P.W
    for c in range(NCORES):
        idx_c = order[pos[c]:pos[c + 1]]
        n_c = len(idx_c)
        assert n_c <= P.EC, f"core {c} has {n_c} edges > {P.EC}"
        srcS = np.zeros(P.EC, np.int64); srcS[:n_c] = src[idx_c]
        dstS = np.zeros(P.EC, np.int64); dstS[:n_c] = dst[idx_c]
        lrS = np.zeros(P.EC, f32); lrS[:n_c] = lr[idx_c]
        dstC = np.full(P.EC, PADDST, f32); dstC[:n_c] = dst[idx_c].astype(f32)
        if n_c > 1:
            runlens = np.diff(np.flatnonzero(
                np.concatenate(([True], dst[idx_c][1:] != dst[idx_c][:-1], [True]))))
            assert runlens.max() <= HA, f"run {runlens.max()} > halo {HA}"

        dmain = dstC.reshape(128, C)
        dstx_np = np.empty((128, W), f32)
        dstx_np[:, HA:HA + C] = dmain
        dstx_np[1:, :HA] = dmain[:-1, C - HA:]
        dstx_np[0, :HA] = -7.0
        dstx_np[:-1, HA + C:] = dmain[1:, :HA]
        dstx_np[127, HA + C:] = -8.0

        seqS = seq_of(srcS.reshape(128, C))
        seqD = seq_of(dstS.reshape(128, C))
        lr_dev = seq_of(lrS.reshape(128, C))

        m = dict(wmap)
        m["zs"] = np.ascontiguousarray(zbfT[:, seqS])
        m["zd"] = np.ascontiguousarray(zbfT[:, seqD])
        m["lrmm"] = np.ascontiguousarray(
            lr_dev.reshape(P.NSC, P.TPSC // 2, 1024)).astype(_bf16np)
        m["dstx"] = dstx_np
        m["alr"] = np.abs(lrS).reshape(128, C).copy()
        in_maps.append(m)
        infos.append((idx_c, n_c))
    return in_maps, infos, E


def kernel(**inputs) -> np.ndarray:
    P = FULL
    in_maps, infos, E = host_prep(P, **inputs)
    if "full" not in _prog_cache:
        _prog_cache["full"] = build_program(P)
    nc = _prog_cache["full"]
    from concourse import bass_utils
    res = bass_utils.run_bass_kernel_spmd(
        nc, in_maps, core_ids=list(range(NCORES)), trace=False)
    out = np.empty(E, np.float32)
    for c in range(NCORES):
        idx_c, n_c = infos[c]
        out[idx_c] = res.results[c]["outb"].ravel()[:n_c]
    return out
